# revision 5
# baseline (speedup 1.0000x reference)
"""FCOS detection-head kernel for 8 Trainium2 NeuronCores (Bass/Tile).

Sharding: core = (img in {0,1}) x (tower in {cls,reg}) x (slab in {top,bottom}).
Every FPN level is split into top/bottom row-slabs with +-5-row recompute
margins so all 8 cores run one identical SPMD program; GroupNorm statistics
are made exact with tiny pair-wise AllReduces (one per layer per level-group).
Convs run as 9-tap shifted float32r matmuls accumulating in PSUM; GN scale /
bias / ReLU are fused into the PSUM-eviction and re-normalization passes.
"""
import sys

sys.path.insert(0, "/opt/trn_rl_repo")

import numpy as np

# ----------------------------------------------------------------------------
# geometry (hardcoded from the problem spec)
# ----------------------------------------------------------------------------
LEV = [(100, 152), (50, 76), (25, 38), (13, 19), (7, 10)]
OWN = [50, 25, 13, 7, 4]          # uniform owned rows per slab
NLAYER = 4
MARG = [4, 3, 2, 1]               # conv_i output row margin beyond owned
HEAD = 80                         # head conv output channels (reg padded to 80)
NL = len(LEV)
DRAMZ = (0, 1)                    # levels whose raw conv output (z) lives in DRAM
EPS = 1e-5
N_CORES = 8
REPL_GROUPS = [[0, 1], [2, 3], [4, 5], [6, 7]]


def _geom():
    g = {}
    g["W"] = [w for (_, w) in LEV]
    g["H"] = [h for (h, _) in LEV]
    g["We"] = [w + (w % 2) for (_, w) in LEV]    # even compute width (fp32r ISA)
    g["Cc"] = [we + 2 for we in g["We"]]         # padded cols
    g["R"] = [o + 10 for o in OWN]               # padded buffer rows
    noff, t = [], 0
    for l in range(NL):
        noff.append(t)
        t += g["R"][l] * g["Cc"][l]
    g["noff"], g["nbuf"] = noff, t
    g["ZR"] = [o + 8 for o in OWN]               # z rows (layer-1 extent)
    g["rg"] = [max(1, 512 // we) for we in g["We"]]
    opos, t = [], 0
    for l in range(NL):
        opos.append(t)
        t += OWN[l] * g["W"][l]
    g["opos"], g["osz"] = opos, t
    return g


G = _geom()


def _plan_level(l, m):
    """Conv tiles and eviction segments for (level, layer-margin)."""
    zlo, zhi = 4 - m, 4 + OWN[l] + m
    own_lo, own_hi = 4, 4 + OWN[l]
    last = own_hi - 1 if l >= 2 else None
    tiles = []
    t0 = zlo
    while t0 < zhi:
        t1 = min(t0 + G["rg"][l], zhi)
        cuts = {t0, t1}
        for cpt in [own_lo, own_hi] + ([last] if last is not None else []):
            if t0 < cpt < t1:
                cuts.add(cpt)
        cs = sorted(cuts)
        segs = []
        for a, b in zip(cs[:-1], cs[1:]):
            is_last = last is not None and a == last and b == last + 1
            acc = is_last or (own_lo <= a < own_hi)
            segs.append((a, b, acc, is_last))
        tiles.append((t0, t1, segs))
        t0 = t1
    nacc = sum(1 for _, _, segs in tiles for s in segs if s[2])
    return tiles, nacc


# ----------------------------------------------------------------------------
# Bass program (one SPMD program for all 8 cores)
# ----------------------------------------------------------------------------
def _build_nc():
    import concourse.bacc as bacc
    import concourse.tile as tile
    from concourse import mybir

    f32, f32r = mybir.dt.float32, mybir.dt.float32r

    nc = bacc.Bacc("TRN2", target_bir_lowering=False, debug=False,
                   num_devices=N_CORES)

    io = {}
    io["canv"] = nc.dram_tensor("canv", [128, 2, G["nbuf"]], f32r,
                                kind="ExternalInput").ap()
    io["wconv"] = nc.dram_tensor("wconv", [NLAYER, 128, 2, 9, 2, 128], f32r,
                                 kind="ExternalInput").ap()
    io["whead"] = nc.dram_tensor("whead", [128, 2, 9, HEAD], f32r,
                                 kind="ExternalInput").ap()
    for nm in ("cbias", "gnw", "gnb"):
        io[nm] = nc.dram_tensor(nm, [128, NLAYER, 2], f32,
                                kind="ExternalInput").ap()
    io["hbias"] = nc.dram_tensor("hbias", [128, 1], f32, kind="ExternalInput").ap()
    io["hfloor"] = nc.dram_tensor("hfloor", [128, 1], f32,
                                  kind="ExternalInput").ap()
    io["gmat"] = nc.dram_tensor("gmat", [128, 16], f32, kind="ExternalInput").ap()
    io["gmat_t"] = nc.dram_tensor("gmat_t", [16, 128], f32,
                                  kind="ExternalInput").ap()
    io["inv_n"] = nc.dram_tensor("inv_n", [16, 4 * NL], f32,
                                 kind="ExternalInput").ap()
    io["msk"] = nc.dram_tensor("msk", [128, NL, 9], f32r,
                               kind="ExternalInput").ap()
    io["flg"] = nc.dram_tensor("flg", [128, NL], f32, kind="ExternalInput").ap()
    io["out"] = nc.dram_tensor("out", [HEAD, G["osz"]], f32,
                               kind="ExternalOutput").ap()

    with tile.TileContext(nc) as tc:
        _emit(tc, nc, io)
    nc.compile()
    return nc


def _emit(tc, nc, io):
    import contextlib
    import concourse.bass as bass
    from concourse import mybir

    f32, f32r = mybir.dt.float32, mybir.dt.float32r
    AF = mybir.ActivationFunctionType
    ALU = mybir.AluOpType
    W, We, Cc, R, ZR, rg, noff = (G["W"], G["We"], G["Cc"], G["R"],
                                  G["ZR"], G["rg"], G["noff"])

    ctx = contextlib.ExitStack()
    singles = ctx.enter_context(tc.tile_pool(name="singles", bufs=1))
    npool = ctx.enter_context(tc.tile_pool(name="npool", bufs=1))
    zspool = ctx.enter_context(tc.tile_pool(name="zspool", bufs=1))
    wpool = ctx.enter_context(tc.tile_pool(name="wpool", bufs=2))
    stg = ctx.enter_context(tc.tile_pool(name="stg", bufs=4))
    sqp = ctx.enter_context(tc.tile_pool(name="sqp", bufs=2))
    ntg = ctx.enter_context(tc.tile_pool(name="ntg", bufs=2))
    hst = ctx.enter_context(tc.tile_pool(name="hst", bufs=3))
    stat = ctx.enter_context(tc.tile_pool(name="stat", bufs=2))
    cps = ctx.enter_context(tc.tile_pool(name="cps", bufs=4, space="PSUM"))
    gps = ctx.enter_context(tc.tile_pool(name="gps", bufs=2, space="PSUM"))
    bps = ctx.enter_context(tc.tile_pool(name="bps", bufs=2, space="PSUM"))
    dram = ctx.enter_context(tc.tile_pool(name="dram", bufs=2, space="DRAM"))

    # ---- constants / parameters in SBUF ----
    NT = [npool.tile([128, 2, R[l], Cc[l]], f32r, tag=f"n{l}", name=f"nt{l}")
          for l in range(NL)]
    for l in range(NL):
        e = noff[l] + R[l] * Cc[l]
        nc.sync.dma_start(out=NT[l].opt(), in_=io["canv"][:, :, noff[l]:e]
                          .rearrange("p h (r c) -> p h r c", c=Cc[l]))

    ZS = {l: zspool.tile([128, 2, ZR[l], W[l]], f32, tag=f"zs{l}", name=f"zst{l}")
          for l in range(NL) if l not in DRAMZ}

    def load_const(name, shape, dt):
        t = singles.tile(shape, dt, name=f"c_{name}")
        nc.sync.dma_start(out=t.opt(), in_=io[name])
        return t

    HWT = load_const("whead", [128, 2, 9, HEAD], f32r)
    CB = load_const("cbias", [128, NLAYER, 2], f32)
    GNW = load_const("gnw", [128, NLAYER, 2], f32)
    GNB = load_const("gnb", [128, NLAYER, 2], f32)
    HB = load_const("hbias", [128, 1], f32)
    FL = load_const("hfloor", [128, 1], f32)
    GM = load_const("gmat", [128, 16], f32)
    GMT = load_const("gmat_t", [16, 128], f32)
    INV = load_const("inv_n", [16, 4 * NL], f32)
    MSK = load_const("msk", [128, NL, 9], f32r)
    FLG = load_const("flg", [128, NL], f32)
    EPS16 = singles.tile([16, 1], f32)
    nc.vector.memset(EPS16.opt(), EPS)

    def mask_bcast_ap(l, j0, j1, wl):
        base = MSK[:, l, j0:j1].opt()
        return bass.AP(tensor=base.tensor, offset=base.offset,
                       ap=[list(base.ap[0]), [0, 2], list(base.ap[1]), [0, wl]])

    # =================== tower layers ===================
    for li in range(NLAYER):
        m = MARG[li]
        plans = {l: _plan_level(l, m) for l in range(NL)}
        # slot layout: contiguous block per (l, h)
        base, off = {}, 0
        for l in range(NL):
            for h in (0, 1):
                base[(l, h)] = off
                off += plans[l][1]
        assert off <= 64, f"slot overflow {off}"

        WT = wpool.tile([128, 2, 9, 2, 128], f32r, tag="wt")
        nc.sync.dma_start(out=WT.opt(), in_=io["wconv"][li])

        SP = stat.tile([128, 64], f32, tag="sp")
        QP = stat.tile([128, 64], f32, tag="qp")
        RH = stat.tile([128, 4 * NL], f32, tag="rh")
        AT = stat.tile([128, NL, 2], f32, tag="at")
        DT = stat.tile([128, NL, 2], f32, tag="dt")
        zdram = {l: dram.tile([128, 2, ZR[l], W[l]], f32, tag=f"zd{l}", name=f"zdr{l}_{li}")
                 for l in DRAMZ}
        last_slot = {}

        def conv_level(l):
            tiles, _ = plans[l]
            wl, wle = W[l], We[l]
            nv = NT[l]
            ctr = {0: 0, 1: 0}
            for (t0, t1, segs) in tiles:
                ncol = (t1 - t0) * wle
                for h in (0, 1):
                    ps = cps.tile([128, 512], f32, tag="cp")
                    pv = ps[:, 0:ncol].rearrange("p (r w) -> p r w", w=wle)
                    k = 0
                    for kh in (0, 1):
                        for dy in (-1, 0, 1):
                            for dx in (-1, 0, 1):
                                t = (dy + 1) * 3 + (dx + 1)
                                rhs = nv[:, kh, t0 + 1 + dy:t1 + 1 + dy,
                                         1 + dx:1 + dx + wle]
                                nc.tensor.matmul(
                                    out=pv, lhsT=WT[:, kh, t, h, :], rhs=rhs,
                                    start=(k == 0), stop=(k == 17))
                                k += 1
                    if l in DRAMZ:
                        zt = stg.tile([128, rg[l] * wl], f32, tag="stg")
                        dst = zt[:, 0:(t1 - t0) * wl].rearrange(
                            "p (r w) -> p r w", w=wl)
                        doff = -t0
                    else:
                        dst = ZS[l][:, h, :, :]
                        doff = 0
                    for (a, b, acc, is_last) in segs:
                        o = dst[:, a + doff:b + doff, :]
                        i_ = pv[:, a - t0:b - t0, 0:wl]
                        if not acc:
                            nc.scalar.activation(out=o, in_=i_, func=AF.Identity,
                                                 bias=CB[:, li, h:h + 1])
                        else:
                            s = base[(l, h)] + ctr[h]
                            ctr[h] += 1
                            if is_last:
                                last_slot[(l, h)] = s
                            nc.scalar.activation(out=o, in_=i_, func=AF.Identity,
                                                 bias=CB[:, li, h:h + 1],
                                                 accum_out=SP[:, s:s + 1])
                            sq = sqp.tile([128, 512], f32, tag="sq")
                            sqv = sq[:, 0:(b - a) * wl].rearrange(
                                "p (r w) -> p r w", w=wl)
                            nc.scalar.activation(out=sqv, in_=o, func=AF.Square,
                                                 accum_out=QP[:, s:s + 1])
                    if l in DRAMZ:
                        nc.sync.dma_start(
                            out=zdram[l][:, h, t0:t1, :],
                            in_=zt[:, 0:(t1 - t0) * wl].rearrange(
                                "p (r w) -> p r w", w=wl))

        def reduce_stats(l):
            nacc = plans[l][1]
            for h in (0, 1):
                if (l, h) in last_slot:
                    s = last_slot[(l, h)]
                    for P in (SP, QP):
                        nc.vector.tensor_scalar(
                            out=P[:, s:s + 1], in0=P[:, s:s + 1],
                            scalar1=FLG[:, l:l + 1], scalar2=None, op0=ALU.mult)
                a = base[(l, h)]
                nc.vector.tensor_reduce(out=RH[:, 4 * l + h:4 * l + h + 1],
                                        in_=SP[:, a:a + nacc],
                                        axis=mybir.AxisListType.X, op=ALU.add)
                nc.vector.tensor_reduce(out=RH[:, 4 * l + 2 + h:4 * l + 3 + h],
                                        in_=QP[:, a:a + nacc],
                                        axis=mybir.AxisListType.X, op=ALU.add)

        def group_allreduce(c0, c1, tag):
            ncols = c1 - c0
            gp = gps.tile([16, 4 * NL], f32, tag="gp")
            nc.tensor.matmul(out=gp[:, 0:ncols], lhsT=GM.opt(),
                             rhs=RH[:, c0:c1], start=True, stop=True)
            gs = stat.tile([16, 4 * NL], f32, tag=f"gs{tag}")
            nc.scalar.copy(out=gs[:, 0:ncols], in_=gp[:, 0:ncols])
            bin_ = dram.tile([16, ncols], f32, tag=f"bi{tag}")
            bout = dram.tile([16, ncols], f32, tag=f"bo{tag}")
            nc.sync.dma_start(out=bin_.opt(), in_=gs[:, 0:ncols])
            nc.gpsimd.collective_compute(
                "AllReduce", ALU.add, replica_groups=REPL_GROUPS,
                ins=[bin_.opt()], outs=[bout.opt()])
            g2 = stat.tile([16, 4 * NL], f32, tag=f"g2{tag}")
            nc.sync.dma_start(out=g2[:, 0:ncols], in_=bout.opt())
            return g2

        def finalize(levels, g2, c0):
            ncols = 4 * len(levels)
            mu = stat.tile([16, 4 * NL], f32, tag="mu")
            nc.vector.tensor_mul(out=mu[:, 0:ncols], in0=g2[:, 0:ncols],
                                 in1=INV[:, c0:c0 + ncols])
            for j, l in enumerate(levels):
                c = 4 * j
                sq16 = stat.tile([16, 2], f32, tag="sq16")
                nc.vector.tensor_mul(out=sq16.opt(), in0=mu[:, c:c + 2],
                                     in1=mu[:, c:c + 2])
                nc.vector.tensor_sub(out=mu[:, c + 2:c + 4],
                                     in0=mu[:, c + 2:c + 4], in1=sq16.opt())
                nc.scalar.activation(out=mu[:, c + 2:c + 4],
                                     in_=mu[:, c + 2:c + 4],
                                     func=AF.Sqrt, bias=EPS16.opt())
                nc.vector.reciprocal(out=mu[:, c + 2:c + 4],
                                     in_=mu[:, c + 2:c + 4])
            bp = bps.tile([128, 4 * NL], f32, tag="bp")
            nc.tensor.matmul(out=bp[:, 0:ncols], lhsT=GMT.opt(),
                             rhs=mu[:, 0:ncols], start=True, stop=True)
            bc = stat.tile([128, 4 * NL], f32, tag="bc")
            nc.scalar.copy(out=bc[:, 0:ncols], in_=bp[:, 0:ncols])
            for j, l in enumerate(levels):
                c = 4 * j
                nc.vector.tensor_mul(out=AT[:, l, :], in0=bc[:, c + 2:c + 4],
                                     in1=GNW[:, li, :])
                ad = stat.tile([128, 2], f32, tag="ad")
                nc.vector.tensor_mul(out=ad.opt(), in0=AT[:, l, :],
                                     in1=bc[:, c:c + 2])
                nc.vector.tensor_sub(out=DT[:, l, :], in0=GNB[:, li, :],
                                     in1=ad.opt())

        def normalize(l):
            zlo, zhi = 4 - m, 4 + OWN[l] + m
            wl = W[l]
            for h in (0, 1):
                if l in DRAMZ:
                    ck = max(1, 1536 // wl)
                    a = zlo
                    while a < zhi:
                        b = min(a + ck, zhi)
                        nt = ntg.tile([128, 1536], f32, tag="ntg")
                        nv_ = nt[:, 0:(b - a) * wl].rearrange(
                            "p (r w) -> p r w", w=wl)
                        nc.sync.dma_start(out=nv_, in_=zdram[l][:, h, a:b, :])
                        nc.scalar.activation(
                            out=NT[l][:, h, a + 1:b + 1, 1:1 + wl], in_=nv_,
                            func=AF.Relu, scale=AT[:, l, h:h + 1],
                            bias=DT[:, l, h:h + 1])
                        a = b
                else:
                    nc.scalar.activation(
                        out=NT[l][:, h, zlo + 1:zhi + 1, 1:1 + wl],
                        in_=ZS[l][:, h, zlo:zhi, :],
                        func=AF.Relu, scale=AT[:, l, h:h + 1],
                        bias=DT[:, l, h:h + 1])
            tb0, tb1 = 5 - m, 5
            bb0, bb1 = 4 + OWN[l], 5 + OWN[l] + m
            nc.vector.tensor_mul(
                out=NT[l][:, :, tb0:tb1, 1:1 + wl],
                in0=NT[l][:, :, tb0:tb1, 1:1 + wl],
                in1=mask_bcast_ap(l, tb0 - 1, tb1 - 1, wl))
            nc.vector.tensor_mul(
                out=NT[l][:, :, bb0:bb1, 1:1 + wl],
                in0=NT[l][:, :, bb0:bb1, 1:1 + wl],
                in1=mask_bcast_ap(l, 4, 5 + m, wl))

        conv_level(0)
        reduce_stats(0)
        g2a = group_allreduce(0, 4, "a")
        for l in range(1, NL):
            conv_level(l)
        for l in range(1, NL):
            reduce_stats(l)
        g2b = group_allreduce(4, 4 * NL, "b")
        finalize([0], g2a, 0)
        normalize(0)
        finalize(list(range(1, NL)), g2b, 4)
        for l in range(1, NL):
            normalize(l)

    # =================== head conv ===================
    for l in range(NL):
        wl, wle = W[l], We[l]
        nv = NT[l]
        t0 = 0
        while t0 < OWN[l]:
            t1 = min(t0 + rg[l], OWN[l])
            nrow = t1 - t0
            ps = cps.tile([128, 512], f32, tag="cp")
            pv = ps[:HEAD, 0:nrow * wle].rearrange("p (r w) -> p r w", w=wle)
            k = 0
            for kh in (0, 1):
                for dy in (-1, 0, 1):
                    for dx in (-1, 0, 1):
                        t = (dy + 1) * 3 + (dx + 1)
                        rhs = nv[:, kh, t0 + 5 + dy:t1 + 5 + dy,
                                 1 + dx:1 + dx + wle]
                        nc.tensor.matmul(out=pv, lhsT=HWT[:, kh, t, :], rhs=rhs,
                                         start=(k == 0), stop=(k == 17))
                        k += 1
            ncol = nrow * wl
            ht = hst.tile([128, 512], f32, tag="hst")
            hv = ht[:HEAD, 0:ncol].rearrange("p (r w) -> p r w", w=wl)
            nc.scalar.activation(out=hv, in_=pv[:, :, 0:wl], func=AF.Identity,
                                 bias=HB[:HEAD, :])
            hv = ht[:HEAD, 0:ncol]
            nc.vector.tensor_scalar(out=hv, in0=hv, scalar1=FL[:HEAD, :],
                                    scalar2=None, op0=ALU.max)
            o0 = G["opos"][l] + t0 * wl
            nc.sync.dma_start(out=io["out"][:, o0:o0 + ncol], in_=hv)
            t0 = t1

    ctx.close()


# ----------------------------------------------------------------------------
# host-side input prep
# ----------------------------------------------------------------------------
def _prep_inputs(inputs):
    f = np.float32
    feats = [np.asarray(inputs[f"feat{i}"], f) for i in range(NL)]
    gm = np.zeros((128, 16), f)
    for p in range(128):
        gm[p, p // 8] = 1.0
    gmt = np.ascontiguousarray(gm.T)
    inv = np.zeros((16, 4 * NL), f)
    for l, (h_, w_) in enumerate(LEV):
        inv[:, 4 * l:4 * l + 4] = 1.0 / (8.0 * h_ * w_)

    def pack_conv_w(w4):   # [4, 256, 256, 3, 3] -> [4, 128, 2, 9, 2, 128]
        w = np.asarray(w4, f)
        r = np.zeros((NLAYER, 128, 2, 9, 2, 128), f)
        for li in range(NLAYER):
            for kh in range(2):
                for oh in range(2):
                    blk = w[li, oh * 128:(oh + 1) * 128,
                            kh * 128:(kh + 1) * 128]        # [o, i, 3, 3]
                    r[li, :, kh, :, oh, :] = blk.transpose(1, 2, 3, 0).reshape(
                        128, 9, 128)
        return r

    def pack_head_w(wh):   # [oc, 256, 3, 3] -> [128, 2, 9, 80]
        w = np.asarray(wh, f)
        oc = w.shape[0]
        r = np.zeros((128, 2, 9, HEAD), f)
        for kh in range(2):
            r[:, kh, :, :oc] = w[:, kh * 128:(kh + 1) * 128].transpose(
                1, 2, 3, 0).reshape(128, 9, oc)
        return r

    reg_parts = [("bbox", 4), ("ctr", 1), ("dim", 3), ("ori", 1), ("kp", 16),
                 ("depth", 1)]
    reg_w_full = np.concatenate([np.asarray(inputs[f"{n}_w"], f)
                                 for n, _ in reg_parts], axis=0)
    cls_hb = np.zeros((128, 1), f)
    cls_hb[:HEAD, 0] = np.asarray(inputs["cls_out_b"], f)
    reg_hb = np.zeros((128, 1), f)
    reg_hb[:26, 0] = np.concatenate([np.asarray(inputs[f"{n}_b"], f)
                                     for n, _ in reg_parts])
    NEG = f(-3.0e38)
    cls_fl = np.full((128, 1), NEG, f)
    reg_fl = np.full((128, 1), NEG, f)
    reg_fl[0:4, 0] = 0.0

    def per_ch(v):          # [4, 256] -> [128, 4, 2]
        a = np.asarray(v, f).reshape(NLAYER, 2, 128)
        return np.ascontiguousarray(a.transpose(2, 0, 1))

    tw = {
        0: dict(wconv=pack_conv_w(inputs["cls_conv_w"]),
                whead=pack_head_w(inputs["cls_out_w"]),
                hbias=cls_hb, hfloor=cls_fl,
                cbias=per_ch(inputs["cls_conv_b"]),
                gnw=per_ch(inputs["cls_gn_w"]), gnb=per_ch(inputs["cls_gn_b"])),
        1: dict(wconv=pack_conv_w(inputs["reg_conv_w"]),
                whead=pack_head_w(reg_w_full),
                hbias=reg_hb, hfloor=reg_fl,
                cbias=per_ch(inputs["reg_conv_b"]),
                gnw=per_ch(inputs["reg_gn_w"]), gnb=per_ch(inputs["reg_gn_b"])),
    }

    in_maps = []
    for core in range(N_CORES):
        img, tower, half = core // 4, (core // 2) % 2, core % 2
        canv = np.zeros((128, 2, G["nbuf"]), f)
        msk = np.ones((128, NL, 9), f)
        flg = np.ones((128, NL), f)
        for l in range(NL):
            h_l, w_l = LEV[l]
            ownstart = 0 if half == 0 else OWN[l]
            r0 = ownstart - 5
            buf = np.zeros((128, 2, G["R"][l], G["Cc"][l]), f)
            glo, ghi = max(0, r0), min(h_l, r0 + G["R"][l])
            if ghi > glo:
                sl = feats[l][img, :, glo:ghi, :]       # [256, rows, W]
                for hh in range(2):
                    buf[:, hh, glo - r0:ghi - r0, 1:1 + w_l] = \
                        sl[hh * 128:(hh + 1) * 128]
            canv[:, :, G["noff"][l]:G["noff"][l] + G["R"][l] * G["Cc"][l]] = \
                buf.reshape(128, 2, -1)
            for j in range(4):                          # top band rows 1..4
                gr = ownstart + (j + 1) - 5
                msk[:, l, j] = 1.0 if 0 <= gr < h_l else 0.0
            for j in range(5):                          # bottom band rows 4+OWN+j
                gr = ownstart + OWN[l] - 1 + j
                msk[:, l, 4 + j] = 1.0 if 0 <= gr < h_l else 0.0
            flg[:, l] = 1.0 if (ownstart + OWN[l] - 1) < h_l else 0.0
        d = tw[tower]
        in_maps.append(dict(
            canv=canv, wconv=d["wconv"], whead=d["whead"], cbias=d["cbias"],
            gnw=d["gnw"], gnb=d["gnb"], hbias=d["hbias"], hfloor=d["hfloor"],
            gmat=gm, gmat_t=gmt, inv_n=inv, msk=msk, flg=flg))
    return in_maps


def _gather_outputs(results):
    f = np.float32
    outs = {}
    for img in range(2):
        for tower in range(2):
            per_level = []
            for l in range(NL):
                h_l, w_l = LEV[l]
                real_bot = h_l - OWN[l]
                o0, sz = G["opos"][l], OWN[l] * w_l
                top = results[img * 4 + tower * 2 + 0]["out"][:, o0:o0 + sz]
                bot = results[img * 4 + tower * 2 + 1]["out"][:, o0:o0 + sz]
                top = top.reshape(HEAD, OWN[l], w_l)
                bot = bot.reshape(HEAD, OWN[l], w_l)[:, :real_bot, :]
                full = np.concatenate([top, bot], axis=1)
                per_level.append(full.reshape(HEAD, h_l * w_l).T)
            outs[(img, tower)] = np.concatenate(per_level, axis=0)
    cls = np.stack([outs[(i, 0)] for i in range(2)]).astype(f)
    reg = np.stack([outs[(i, 1)] for i in range(2)])
    return (cls,
            reg[:, :, 0:4].astype(f), reg[:, :, 4:5].astype(f),
            reg[:, :, 5:8].astype(f), reg[:, :, 8:9].astype(f),
            reg[:, :, 9:25].astype(f), reg[:, :, 25:26].astype(f))


_NC_CACHE = []


def _get_nc():
    if not _NC_CACHE:
        _NC_CACHE.append(_build_nc())
    return _NC_CACHE[0]


def kernel(**inputs):
    from concourse import bass_utils
    nc = _get_nc()
    in_maps = _prep_inputs(inputs)
    res = bass_utils.run_bass_kernel_spmd(nc, in_maps,
                                          core_ids=list(range(N_CORES)))
    return _gather_outputs(res.results)
